# revision 1
# baseline (speedup 1.0000x reference)
"""Ewald reciprocal-space sum on 8 Trainium2 NeuronCores.

Math: for each system b, S(k) = sum_n q_n e^{i k.r_n} over the static
integer k-grid n in [-10,10]^3, k = n @ G, G = 2*pi*inv(cell)^T.
Key identity: k.r = n1*phi1 + n2*phi2 + n3*phi3 with phi_d = G_d . r,
so e^{i k.r} factorizes into per-dimension phase tables. Only the
n1 >= 0 half-grid is needed (hemisphere mask kills n1 < 0).

Device work per core (SPMD, core c owns half the atoms of system c//2):
  - phases phi'_d = frac((r @ inv(cell))_d) come in pre-reduced (turns)
  - theta'[j,d] = j * phi'_d  for j in [-10..10]        (DVE)
  - reduce mod 1 into [0,1) turn space                  (DVE/GPSIMD)
  - sin/cos via ACT Sin(2*pi*t - pi)                    (ACT)
  - pair table A = e^{i(n2*phi2+n3*phi3)}  [atoms,441]  (ACT)
  - S partial = (q*e^{i n1 phi1})^T @ A via 2 PSUM-accumulated
    matmuls per 128-atom chunk                          (PE)
Host: tiny O(B*K) weight mask + final reduction (exactly mirrors the
reference), summing partial S across the core pair before squaring.
"""

import numpy as np

# ---- problem constants (hardcoded per contract) ----
B = 4
N_PER = 2000
NK = 10                      # k-grid extent: n in [-NK, NK]
NJ = 2 * NK + 1              # 21
NPAIR = NJ * NJ              # 441
NH = NK + 1                  # 11 non-negative n1 values
DL = 2.0
SIGMA = 1.0
EPS = 1e-6
NORM = 90.0474
TWOPI = 2.0 * np.pi

MAGIC = 12582912.0           # 1.5 * 2**23: fp32 round-to-nearest trick
NPP = NPAIR + 1              # 442: fp32r matmul needs even free counts

N_CORES = 8
CORES_PER_SYS = 2
ATOMS_PER_CORE = (B * N_PER) // N_CORES     # 1000
CHUNKS = 8                                  # ceil(1000/128)
PADN = CHUNKS * 128                         # 1024

_CACHE = {}


def _build_nc():
    import concourse.bacc as bacc
    import concourse.mybir as mybir
    import concourse.tile as tile

    # cheaper TileContext exit: the Bass preamble re-clears the whole
    # kernel sem range at every execution, so the exit-time sem clear and
    # second all-engine barrier are redundant for this single-context
    # kernel; keep drain + one barrier.
    def _cheap_drain_and_barrier(self, tick_clock, wait_clock):
        drain_inst = self.nc.sync.drain()
        wait_clock.add_sem_waits(
            drain_inst.ins, tile.ScopedClock({None: tick_clock.global_clock})
        )
        popped = self.nc._tile_sem_poison_stack.pop()
        assert popped is self._sem_poison

    f32 = mybir.dt.float32
    Alu = mybir.AluOpType
    Act = mybir.ActivationFunctionType

    # fused custom DVE op: out = wrap(in0 + in1 + s0) into [-s1, s1] with
    # period 1 (turn space) -- replaces gpsimd add + add_range_wrap pair
    import concourse.dve_ops as dve_ops

    if not hasattr(dve_ops, "ADD_WRAP_EWALD"):
        from concourse.dve_spec import C0, C1, Spec, Src0, Src1, lower
        from concourse.dve_uop import DveOpSpec

        _y = (Src0 + Src1) + C0

        def _ref(in0, in1, s0, s1, imm2):
            y = in0 + in1 + s0
            return y + (
                (y < -s1).astype(np.float32) - (y > s1).astype(np.float32)
            )

        _spec = Spec(body=_y + ((_y < -C1) - (_y > C1)), reference=_ref)
        _shas = {
            ver: DveOpSpec(
                name="ADD_WRAP_EWALD", opcode=0,
                uops=lower(_spec, ver=ver), rd1_en=True,
            ).sha(ver)
            for ver in ("v3", "v4")
        }
        _op = dve_ops.DveOp("ADD_WRAP_EWALD", _spec, subdim=False, uops_sha=_shas)
        dve_ops.OPS.append(_op)
        dve_ops._SUB_OPCODE_FOR_NAME[_op.name] = (
            dve_ops._CUSTOM_DVE_ROW_BASE + len(dve_ops.OPS) - 1
        )
        dve_ops.CUSTOM_DVE_SPECS[_op.name] = _spec
        dve_ops.ADD_WRAP_EWALD = _op
    AW = dve_ops.ADD_WRAP_EWALD

    f32r = mybir.dt.float32r
    tile.TileContext._drain_and_barrier = _cheap_drain_and_barrier
    nc = bacc.Bacc(None, target_bir_lowering=False)

    # one input tensor: cols 0:24 = phi (chunk-major, 3/chunk), 24:32 = q
    inp = nc.dram_tensor("inp", [128, 3 * CHUNKS + CHUNKS], f32, kind="ExternalInput")
    sout = nc.dram_tensor("sout", [2 * NH, 2 * NPP], f32, kind="ExternalOutput")

    # j values (d-major blocks of 21, col = d*21 + j+10) + col 63 = -2pi
    jdat = np.concatenate(
        [
            np.tile(np.arange(-NK, NK + 1, dtype=np.float32), (128, 3)),
            np.full((128, 1), -TWOPI, np.float32),
        ],
        axis=1,
    )
    jrow = nc.inline_tensor(jdat, name="jrow")

    NW = 3 * NJ                      # 63 cols per chunk in F/th tiles
    NV = 4 * NPP                     # 1768 cols in fused ACT input per pair
    NT = 2 * CHUNKS * NH             # 176 cols of d1 tables (k-major: c1|s1)

    with tile.TileContext(nc) as tc:
        with (
            tc.tile_pool(name="const", bufs=1) as cp,
            tc.tile_pool(name="work", bufs=3) as wp,
            tc.tile_pool(name="psum", bufs=1, space="PSUM") as pp,
        ):
            it = cp.tile([128, 4 * CHUNKS], f32)
            nc.gpsimd.dma_start(out=it[:], in_=inp[:])
            jt = cp.tile([128, NW + 1], f32)
            nc.sync.dma_start(out=jt[:], in_=jrow[:])
            cm2pi = jt[:, NW : NW + 1]

            ps_r = pp.tile([2 * NH, NPP], f32)
            ps_i = pp.tile([2 * NH, NPP], f32)

            # stage 0 in two halves: theta' = j*phi'; F = round - theta'
            HC = CHUNKS // 2
            tha = cp.tile([128, CHUNKS * NW], f32)
            t1a = cp.tile([128, CHUNKS * NW], f32)
            Fa = cp.tile([128, CHUNKS * NW], f32)
            for h in range(2):
                hs, he = h * HC * NW, (h + 1) * HC * NW
                nc.vector.tensor_tensor(
                    out=tha[:, hs:he].rearrange("p (t d j) -> p t d j", t=HC, d=3),
                    in0=it[:, 3 * h * HC : 3 * (h + 1) * HC]
                    .rearrange("p (t d) -> p t d", d=3)
                    .unsqueeze(3)
                    .broadcast_to([128, HC, 3, NJ]),
                    in1=jt[:, 0:NW]
                    .rearrange("p (d j) -> p d j", d=3)
                    .unsqueeze(1)
                    .broadcast_to([128, HC, 3, NJ]),
                    op=Alu.mult,
                )
                nc.vector.tensor_scalar(
                    out=t1a[:, hs:he], in0=tha[:, hs:he], scalar1=MAGIC,
                    scalar2=None, op0=Alu.add,
                )
                nc.vector.scalar_tensor_tensor(
                    out=Fa[:, hs:he], in0=t1a[:, hs:he], scalar=-MAGIC,
                    in1=tha[:, hs:he], op0=Alu.add, op1=Alu.subtract,
                )

            Fv = Fa[:].rearrange("p (t w) -> p t w", t=CHUNKS)  # [128, 8, 63]

            # d1 tables, (t, k, j) interleaved: cols 22t+j = c1, 22t+11+j = s1
            F1a = Fv[:, :, NK : NK + NH]                       # [128, 8, 11]
            VT = cp.tile([128, NT], f32)
            TT = cp.tile([128, NT], f32)
            lhsTa = cp.tile([128, NT], f32r)
            tkj = lambda ap: ap.rearrange("p (t k j) -> p t k j", t=CHUNKS, k=2)
            nc.vector.add_range_wrap(
                out=tkj(VT[:])[:, :, 0, :], in_=F1a, shift=-0.25,
                bound=0.5, period=1.0,
            )
            nc.scalar.activation(
                out=tkj(TT[:])[:, :, 1, :], in_=F1a, func=Act.Sin,
                bias=0.0, scale=cm2pi,
            )
            nc.scalar.activation(
                out=tkj(TT[:])[:, :, 0, :], in_=tkj(VT[:])[:, :, 0, :],
                func=Act.Sin, bias=0.0, scale=cm2pi,
            )
            nc.gpsimd.tensor_tensor(
                out=tkj(lhsTa[:]),
                in0=tkj(TT[:]),
                in1=it[:, 3 * CHUNKS : 4 * CHUNKS]
                .unsqueeze(2)
                .unsqueeze(3)
                .broadcast_to([128, CHUNKS, 2, NH]),
                op=Alu.mult,
            )

            for t in range(CHUNKS):
                F2bc = Fv[:, t, NJ : 2 * NJ].unsqueeze(2).broadcast_to(
                    [128, NJ, NJ]
                )
                F3bc = Fv[:, t, 2 * NJ : 3 * NJ].unsqueeze(1).broadcast_to(
                    [128, NJ, NJ]
                )
                # V = [A_i src (442) | A_r src (442)]; col 441 of each
                # block is pad (fp32r needs even counts); host ignores it.
                # Each block = wrap(F2 (+) F3 + shift) fused in one DVE op.
                V = wp.tile([128, 2 * NPP], f32)
                Vb = V[:].rearrange("p (blk w) -> p blk w", blk=2)
                nc.gpsimd.memset(Vb[:, :, NPAIR:NPP], 0.0)
                nc.vector._custom_dve(
                    AW, out=Vb[:, 0, 0:NPAIR].rearrange("p (a b) -> p a b", a=NJ),
                    in0=F2bc, in1=F3bc, s0=0.0, s1=0.5,
                )
                nc.vector._custom_dve(
                    AW, out=Vb[:, 1, 0:NPAIR].rearrange("p (a b) -> p a b", a=NJ),
                    in0=F2bc, in1=F3bc, s0=-0.25, s1=0.5,
                )
                # Sin(-2pi*v) -> [A_i | A_r]
                AA = wp.tile([128, 2 * NPP], f32r)
                nc.scalar.activation(
                    out=AA[:], in_=V[:], func=Act.Sin, bias=0.0, scale=cm2pi
                )
                lh = lhsTa[:, 2 * NH * t : 2 * NH * (t + 1)]
                nc.tensor.matmul(
                    out=ps_i[:], lhsT=lh, rhs=AA[:, 0:NPP],
                    start=(t == 0), stop=(t == CHUNKS - 1),
                )
                nc.tensor.matmul(
                    out=ps_r[:], lhsT=lh, rhs=AA[:, NPP : 2 * NPP],
                    start=(t == 0), stop=(t == CHUNKS - 1),
                )

            # PSUM -> SBUF -> DRAM (combine happens on host)
            so = wp.tile([2 * NH, 2 * NPP], f32)
            nc.vector.tensor_copy(out=so[:, 0:NPP], in_=ps_r[:])
            nc.scalar.activation(
                out=so[:, NPP : 2 * NPP], in_=ps_i[:], func=Act.Copy
            )
            nc.sync.dma_start(out=sout[:, 0:NPP], in_=so[:, 0:NPP])
            nc.sync.dma_start(out=sout[:, NPP : 2 * NPP], in_=so[:, NPP : 2 * NPP])

    nc.compile()
    return nc


def _get_nc():
    if "nc" not in _CACHE:
        _CACHE["nc"] = _build_nc()
    return _CACHE["nc"]


def _host_inputs(q, r, cell):
    """Per-core phi (reduced turns) and q in SBUF layout."""
    in_maps = []
    for c in range(N_CORES):
        b = c // CORES_PER_SYS
        half = c % CORES_PER_SYS
        lo = b * N_PER + half * ATOMS_PER_CORE
        rs = r[lo : lo + ATOMS_PER_CORE].astype(np.float64)
        qs = q[lo : lo + ATOMS_PER_CORE, 0].astype(np.float32)
        minv = np.linalg.inv(cell[b].astype(np.float64))
        phi = (rs @ minv) % 1.0                      # [1000, 3] turns in [0,1)
        phi_p = np.zeros((PADN, 3), np.float32)
        phi_p[:ATOMS_PER_CORE] = phi.astype(np.float32)
        q_p = np.zeros((PADN,), np.float32)
        q_p[:ATOMS_PER_CORE] = qs
        # atom (t*128+p) -> [p, t*3+d] and [p, 24+t]
        inp = np.zeros((128, 4 * CHUNKS), np.float32)
        inp[:, 0 : 3 * CHUNKS] = (
            phi_p.reshape(CHUNKS, 128, 3).transpose(1, 0, 2).reshape(128, CHUNKS * 3)
        )
        inp[:, 3 * CHUNKS :] = q_p.reshape(CHUNKS, 128).T
        in_maps.append({"inp": inp})
    return in_maps


def _host_weights(cell):
    """w[b, n1(0..10), n2, n3] = mask * 2 * kfac / V, mirroring reference."""
    k_sq_max = (TWOPI / DL) ** 2
    sigma_sq_half = SIGMA ** 2 / 2.0
    rng = np.arange(-NK, NK + 1, dtype=np.float64)
    n1, n2, n3 = np.meshgrid(rng[NK:], rng, rng, indexing="ij")  # n1 >= 0
    nvec = np.stack([n1.ravel(), n2.ravel(), n3.ravel()], axis=1)  # [NH*441, 3]
    hemi = (
        (nvec[:, 0] > 0)
        | ((nvec[:, 0] == 0) & (nvec[:, 1] > 0))
        | ((nvec[:, 0] == 0) & (nvec[:, 1] == 0) & (nvec[:, 2] > 0))
    )
    ws = []
    for b in range(B):
        cb = cell[b].astype(np.float64)
        G = TWOPI * np.linalg.inv(cb).T
        kvec = nvec @ G
        k_sq = np.sum(kvec ** 2, axis=1)
        mask = (k_sq > 0) & (k_sq <= k_sq_max) & hemi
        kfac = np.exp(-sigma_sq_half * k_sq) / (k_sq + EPS)
        vol = np.linalg.det(cb)
        ws.append(np.where(mask, 2.0 * kfac, 0.0) / vol)
    return np.stack(ws).reshape(B, NH, NPAIR)


def kernel(q, r, cell, batch):
    from concourse.bass_utils import run_bass_kernel_spmd

    q = np.asarray(q)
    r = np.asarray(r)
    cell = np.asarray(cell)

    nc = _get_nc()
    in_maps = _host_inputs(q, r, cell)
    res = run_bass_kernel_spmd(nc, in_maps, core_ids=list(range(N_CORES))).results

    w = _host_weights(cell)
    pot = np.zeros(B, np.float64)
    for b in range(B):
        s_r = np.zeros((NH, NPAIR), np.float64)
        s_i = np.zeros_like(s_r)
        for half in range(CORES_PER_SYS):
            o = res[b * CORES_PER_SYS + half]["sout"].astype(np.float64)
            P, Q = o[0:NH, 0:NPAIR], o[NH : 2 * NH, 0:NPAIR]
            R, T = o[0:NH, NPP : NPP + NPAIR], o[NH : 2 * NH, NPP : NPP + NPAIR]
            s_r += P - T
            s_i += R + Q
        s_sq = s_r ** 2 + s_i ** 2
        qb = q[b * N_PER : (b + 1) * N_PER, 0].astype(np.float64)
        self_e = np.sum(qb ** 2) / (SIGMA * TWOPI ** 1.5)
        pot[b] = (np.sum(w[b] * s_sq) - self_e) * NORM
    return pot.astype(np.float32)



# revision 2
# speedup vs baseline: 1.5132x; 1.5132x over previous
"""Ewald reciprocal-space sum on 8 Trainium2 NeuronCores.

Math: for each system b, S(k) = sum_n q_n e^{i k.r_n} over the integer
k-grid n in [-10,10]^3, k = n @ G, G = 2*pi*inv(cell)^T. The weight mask
keeps only k_sq <= (2*pi/DL)^2, i.e. |n|^2 <= 100, and one hemisphere.

Factorization used here: k.r = 2*pi*(n1*phi1 + n2*phi2 + n3*phi3) with
phi_d = (r @ inv(cell))_d, so
  S[n1,n2,n3] = sum_a (q_a e^{i 2pi n3 phi3}) * e^{i 2pi(n1 phi1 + n2 phi2)}.
The (n1,n2) pair table (n1 in [0,10] half-grid, sphere-pruned to two
rectangular blocks, 207 pairs x {sin,cos} = 414 cols) is built on device:
one fused DVE add+wrap per block per 128-atom chunk, then one ACT Sin,
then one PE matmul per chunk against the 42-col stationary side
qv = [-q sin(2pi n3 phi3) | q cos(2pi n3 phi3)], accumulating
PSUM[42, 414] over the 8 chunks.

Host does O(N) prep (centered fractional phases frac(j*phi), the qv
table) and the tiny O(K) weighted reduction mirroring the reference,
summing partial S across the core pair of each system before squaring.
Each core owns half the atoms of system c//2.
"""

import numpy as np

# ---- problem constants (hardcoded per contract) ----
B = 4
N_PER = 2000
NK = 10
DL = 2.0
SIGMA = 1.0
EPS = 1e-6
NORM = 90.0474
TWOPI = 2.0 * np.pi

N_CORES = 8
CORES_PER_SYS = 2
ATOMS_PER_CORE = (B * N_PER) // N_CORES     # 1000
CHUNKS = 8                                  # ceil(1000/128)
PADN = CHUNKS * 128                         # 1024

# (n1_lo, n1_hi, n2_lo, n2_hi) inclusive; pairs outside |n|^2<=100 never
# survive the reference's k_sq mask, so the corner blocks are dropped.
BLOCKS = [(0, 6, -10, 10), (7, 10, -7, 7)]
NP_PAIRS = sum((a1 - a0 + 1) * (b1 - b0 + 1) for a0, a1, b0, b1 in BLOCKS)
NCOL = 2 * NP_PAIRS                         # 414: (n1,h={sin,cos}) x n2
CPC = 85                                    # input cols/chunk: f1x 22|f2c 21|qv 42
NIN = CHUNKS * CPC                          # 680
# |scale|*0.5 must stay <= pi in fp32 (ACT Sin domain); 6.283185 < 2*pi
SCALE = -6.283185

_CACHE = {}


def _build_nc():
    import concourse.bacc as bacc
    import concourse.mybir as mybir
    import concourse.tile as tile

    # cheaper TileContext exit: the Bass preamble re-clears the whole
    # kernel sem range at every execution, so the exit-time sem clear and
    # second all-engine barrier are redundant for this single-context
    # kernel; keep drain + one barrier.
    def _cheap_drain_and_barrier(self, tick_clock, wait_clock):
        drain_inst = self.nc.sync.drain()
        wait_clock.add_sem_waits(
            drain_inst.ins, tile.ScopedClock({None: tick_clock.global_clock})
        )
        popped = self.nc._tile_sem_poison_stack.pop()
        assert popped is self._sem_poison

    f16 = mybir.dt.float16
    f32 = mybir.dt.float32
    Act = mybir.ActivationFunctionType

    # fused custom DVE op: out = wrap(in0 + in1 + s0) into [-s1, s1] with
    # period 1 (turn space)
    import concourse.dve_ops as dve_ops

    if not hasattr(dve_ops, "ADD_WRAP_EWALD"):
        from concourse.dve_spec import C0, C1, Spec, Src0, Src1, lower
        from concourse.dve_uop import DveOpSpec

        _y = (Src0 + Src1) + C0

        def _ref(in0, in1, s0, s1, imm2):
            y = in0 + in1 + s0
            return y + (
                (y < -s1).astype(np.float32) - (y > s1).astype(np.float32)
            )

        _spec = Spec(body=_y + ((_y < -C1) - (_y > C1)), reference=_ref)
        _shas = {
            ver: DveOpSpec(
                name="ADD_WRAP_EWALD", opcode=0,
                uops=lower(_spec, ver=ver), rd1_en=True,
            ).sha(ver)
            for ver in ("v3", "v4")
        }
        _op = dve_ops.DveOp("ADD_WRAP_EWALD", _spec, subdim=False, uops_sha=_shas)
        dve_ops.OPS.append(_op)
        dve_ops._SUB_OPCODE_FOR_NAME[_op.name] = (
            dve_ops._CUSTOM_DVE_ROW_BASE + len(dve_ops.OPS) - 1
        )
        dve_ops.CUSTOM_DVE_SPECS[_op.name] = _spec
        dve_ops.ADD_WRAP_EWALD = _op
    AW = dve_ops.ADD_WRAP_EWALD

    tile.TileContext._drain_and_barrier = _cheap_drain_and_barrier
    nc = bacc.Bacc(None, target_bir_lowering=False)

    inp = nc.dram_tensor("inp", [128, NIN], f16, kind="ExternalInput")
    sout = nc.dram_tensor("sout", [42, NCOL], f16, kind="ExternalOutput")

    HALF = NIN // 2

    with tile.TileContext(nc) as tc:
        with (
            tc.tile_pool(name="const", bufs=1) as cp,
            tc.tile_pool(name="work", bufs=3) as wp,
            tc.tile_pool(name="psum", bufs=1, space="PSUM") as pp,
        ):
            INa = cp.tile([128, HALF], f16)
            nc.sync.dma_start(out=INa[:], in_=inp[:, 0:HALF])
            INb = cp.tile([128, HALF], f16)
            nc.gpsimd.dma_start(out=INb[:], in_=inp[:, HALF:NIN])

            ps = pp.tile([42, NCOL], f32)

            for t in range(CHUNKS):
                IN = INa if t < 4 else INb
                base = (t % 4) * CPC
                usrc = wp.tile([128, NCOL], f16)
                off = 0
                for a0, a1, b0, b1 in BLOCKS:
                    na, nb = a1 - a0 + 1, b1 - b0 + 1
                    nc.vector._custom_dve(
                        AW,
                        out=usrc[:, off : off + 2 * na * nb].rearrange(
                            "p (ah b) -> p ah b", b=nb
                        ),
                        in0=IN[:, base + 2 * a0 : base + 2 * a1 + 2]
                        .unsqueeze(2)
                        .broadcast_to([128, 2 * na, nb]),
                        in1=IN[:, base + 22 + b0 + 10 : base + 22 + b1 + 11]
                        .unsqueeze(1)
                        .broadcast_to([128, 2 * na, nb]),
                        s0=0.0, s1=0.5,
                    )
                    off += 2 * na * nb
                AA = wp.tile([128, NCOL], f16)
                nc.scalar.activation(
                    out=AA[:], in_=usrc[:], func=Act.Sin, bias=0.0, scale=SCALE
                )
                nc.tensor.matmul(
                    out=ps[:], lhsT=IN[:, base + 43 : base + CPC], rhs=AA[:],
                    start=(t == 0), stop=(t == CHUNKS - 1),
                )

            so = wp.tile([42, NCOL], f16)
            nc.scalar.activation(out=so[:], in_=ps[:], func=Act.Copy)
            nc.sync.dma_start(out=sout[:], in_=so[:])

    nc.compile()
    return nc


def _get_nc():
    if "nc" not in _CACHE:
        _CACHE["nc"] = _build_nc()
    return _CACHE["nc"]


def _cf(x):
    """centered frac: ((x+0.5) mod 1) - 0.5 in [-0.5, 0.5)"""
    return ((x + 0.5) % 1.0) - 0.5


def _host_inputs(q, r, cell):
    """Per-core phase/qv tables in SBUF layout, fp16."""
    j1 = np.arange(11)
    n2r = np.arange(-10, 11)
    n3r = np.arange(-10, 11)
    in_maps = []
    for c in range(N_CORES):
        b = c // CORES_PER_SYS
        half = c % CORES_PER_SYS
        lo = b * N_PER + half * ATOMS_PER_CORE
        rs = r[lo : lo + ATOMS_PER_CORE].astype(np.float64)
        qs = q[lo : lo + ATOMS_PER_CORE, 0].astype(np.float64)
        minv = np.linalg.inv(cell[b].astype(np.float64))
        phi = rs @ minv                     # turns (unwrapped)
        dat = np.zeros((ATOMS_PER_CORE, CPC))
        p1 = np.outer(phi[:, 0], j1)
        dat[:, 0:22:2] = _cf(p1)            # h=0: sin-src
        dat[:, 1:22:2] = _cf(p1 - 0.25)     # h=1: cos-src
        dat[:, 22:43] = _cf(np.outer(phi[:, 1], n2r))
        gam = TWOPI * np.outer(phi[:, 2], n3r)
        dat[:, 43:64] = -qs[:, None] * np.sin(gam)
        dat[:, 64:85] = qs[:, None] * np.cos(gam)
        dat_p = np.zeros((PADN, CPC), np.float16)
        dat_p[:ATOMS_PER_CORE] = dat.astype(np.float16)
        # atom (t*128+p) -> row p, cols [t*CPC : (t+1)*CPC]
        inp = (
            dat_p.reshape(CHUNKS, 128, CPC).transpose(1, 0, 2).reshape(128, NIN)
        )
        in_maps.append({"inp": inp})
    return in_maps


def _host_weights(cell):
    """w[b, pair, n3] mirroring the reference's fp32 mask/kfac semantics."""
    k_sq_max = np.float32((TWOPI / DL) ** 2)
    ssh = np.float32(SIGMA ** 2 / 2.0)
    pairs = []
    for a0, a1, b0, b1 in BLOCKS:
        for n1 in range(a0, a1 + 1):
            for n2 in range(b0, b1 + 1):
                pairs.append((n1, n2))
    pairs = np.array(pairs)
    nvec = np.zeros((NP_PAIRS, 21, 3), np.float32)
    nvec[:, :, 0] = pairs[:, 0:1]
    nvec[:, :, 1] = pairs[:, 1:2]
    nvec[:, :, 2] = np.arange(-10, 11)[None, :]
    nflat = nvec.reshape(-1, 3)
    hemi = (
        (nflat[:, 0] > 0)
        | ((nflat[:, 0] == 0) & (nflat[:, 1] > 0))
        | ((nflat[:, 0] == 0) & (nflat[:, 1] == 0) & (nflat[:, 2] > 0))
    )
    ws = []
    for b in range(B):
        cb = cell[b]
        G = (np.float32(TWOPI) * np.linalg.inv(cb.astype(np.float64)).T).astype(
            np.float32
        )
        kvec = (nflat @ G).astype(np.float32)
        k_sq = np.sum(kvec * kvec, axis=1, dtype=np.float32)
        mask = (k_sq > 0) & (k_sq <= k_sq_max) & hemi
        kfac = np.exp(-ssh * k_sq) / (k_sq + np.float32(EPS))
        vol = np.float32(np.linalg.det(cb.astype(np.float64)))
        ws.append(np.where(mask, 2.0 * kfac, 0.0).astype(np.float64) / vol)
    return np.stack(ws).reshape(B, NP_PAIRS, 21)


def _col_maps():
    sin_col = np.zeros(NP_PAIRS, np.int64)
    cos_col = np.zeros(NP_PAIRS, np.int64)
    off = p = 0
    for a0, a1, b0, b1 in BLOCKS:
        na, nb = a1 - a0 + 1, b1 - b0 + 1
        for a in range(na):
            for bb in range(nb):
                sin_col[p] = off + (2 * a) * nb + bb
                cos_col[p] = off + (2 * a + 1) * nb + bb
                p += 1
        off += 2 * na * nb
    return sin_col, cos_col


def kernel(q, r, cell, batch):
    from concourse.bass_utils import run_bass_kernel_spmd

    q = np.asarray(q)
    r = np.asarray(r)
    cell = np.asarray(cell)

    nc = _get_nc()
    in_maps = _host_inputs(q, r, cell)
    res = run_bass_kernel_spmd(nc, in_maps, core_ids=list(range(N_CORES))).results

    w = _host_weights(cell)
    sin_col, cos_col = _col_maps()
    pot = np.zeros(B, np.float64)
    for b in range(B):
        P = (
            res[2 * b]["sout"].astype(np.float64)
            + res[2 * b + 1]["sout"].astype(np.float64)
        )
        S_r = P[21:42, :][:, cos_col].T - P[0:21, :][:, sin_col].T
        S_i = -P[21:42, :][:, sin_col].T - P[0:21, :][:, cos_col].T
        s_sq = S_r ** 2 + S_i ** 2
        qb = q[b * N_PER : (b + 1) * N_PER, 0].astype(np.float64)
        self_e = np.sum(qb ** 2) / (SIGMA * TWOPI ** 1.5)
        pot[b] = (np.sum(w[b] * s_sq) - self_e) * NORM
    return pot.astype(np.float32)
